# revision 9
# baseline (speedup 1.0000x reference)
"""Block-diagonal linear y = x @ W_blockdiag.T + bias on 8 TRN2 NeuronCores.

Expert-parallel sharding: core k owns diagonal block k — x[:, 512k:512(k+1)],
weight_blocks[k] (512x512), bias[512k:512(k+1)] — and produces the matching
output column slice y[:, 512k:512(k+1)]. No collectives.

v3: host pre-transposes/casts everything to fp16 so the device kernel is
pure matmul + bias-add evacuation, and DMA is split across both HWDGE
rings (each ring sustains only ~210 GB/s — the 360 GB/s core figure needs
both):
  - x.T arrives with the contraction dim c on partitions: NO on-chip
    transposes; y.T [512, 8192] is computed directly (r on partitions)
  - x loads split by c-chunk: ci 0-1 on the SP ring, ci 2-3 on the ACT
    ring -> all of x is resident by ~15us
  - y.T stores split by r-chunk: rj 0-1 on SP, rj 2-3 on ACT, one store
    per (group, rj) with 2-4KB descriptor lines; they queue naturally
    behind the x loads in each ring's FIFO
  - matmuls run in ns-groups (GROUPS) so one stationary W.T block serves
    the whole group (fewer LDWEIGHTS stalls); PSUM banks j of a group
    accumulate 4 c-chunks each, start/stop per bank
  - bias add + fp16 cast fused into the PSUM->SBUF evac, alternating
    DVE/ACT per bank
  - PE warm-up: transposes of a DVE-memset zero tile (no GpSimd identity
    chain) flip the HAM clock gate while the first DMAs land
  - fp16 end-to-end keeps rel err ~3e-4 (tolerance 2e-2); DMA per core:
    8.4 MB in + 8.4 MB out; PE floor 131072 cyc @ 2.4 GHz = 54.6 us
"""

import os
import sys

import numpy as np

for _p in ("/opt/trn_rl_repo", "/root/.axon_site/_ro/trn_rl_repo"):
    if os.path.isdir(_p) and _p not in sys.path:
        sys.path.insert(0, _p)

import concourse.bass as bass
import concourse.mybir as mybir
import concourse.tile as tile
from concourse.bass_utils import run_bass_kernel_spmd
from concourse.tile_rust import add_dep_helper

# Problem shape (hardcoded per spec nn_BlockDiagLinear_19490561590005)
N = 8192          # tokens
D = 4096          # model dim
NB = 8            # diagonal blocks == number of cores
B = 512           # block size (rows == cols)
P = 128           # SBUF partitions
CB = B // P       # 4 contraction chunks of 128
RB = B // P       # 4 output-row chunks of 128
SLICE = 512       # moving free dim per matmul == one PSUM bank of fp32
NS = N // SLICE   # 16 token slices

F32 = mybir.dt.float32
F16 = mybir.dt.float16

# compute/store ns-groups: small last groups = short drain tail
GROUPS = [2, 2, 4, 4, 2, 1, 1]
assert sum(GROUPS) == NS
# x-load ns-groups: single slices first (fast pipeline fill), 4KB lines later
XGROUPS = [1, 1, 1, 1, 2, 2, 4, 4]
assert sum(XGROUPS) == NS
WARMUP_MATMULS = 10  # ~3-4us of PE busy -> HAM clock gate open early

_CACHE = {}


def _build_bass():
    nc = bass.Bass("TRN2", target_bir_lowering=False)
    xt_d = nc.dram_tensor("xt", [B, N], F16, kind="ExternalInput")   # x.T slice
    wt_d = nc.dram_tensor("wt", [B, B], F16, kind="ExternalInput")   # W.T
    b_d = nc.dram_tensor("b", [P, RB], F32, kind="ExternalInput")    # bias, p-major
    y_d = nc.dram_tensor("y", [B, N], F16, kind="ExternalOutput")    # y.T slice

    # ring assignment: ci/rj 0-1 -> SP, 2-3 -> ACT
    def ring(i):
        return nc.sync if i < 2 else nc.scalar

    with tile.TileContext(nc) as tc:
        with (
            tc.tile_pool(name="const", bufs=1) as const_pool,
            tc.tile_pool(name="psY", bufs=7, space="PSUM") as psY_pool,
            tc.tile_pool(name="psD", bufs=1, space="PSUM") as psD_pool,
        ):
            # W.T chunks: wt_sb[:, ci*512 + r] = W[r, ci*128 + c]
            wt_sb = const_pool.tile([P, CB * B], F16)
            # x.T chunks: xt_sb[ci][c, n] = x[n, ci*128 + c]
            xt_sb = [
                const_pool.tile([P, N], F16, name=f"xt_sb{ci}") for ci in range(CB)
            ]
            # y.T chunks: yt_sb[rj][r, n] = y[n, rj*128 + r]
            yt_sb = [
                const_pool.tile([P, N], F16, name=f"yt_sb{rj}") for rj in range(RB)
            ]
            bias_sb = const_pool.tile([P, RB], F32)

            def load_x_group(ns0, g):
                for ci in range(CB):
                    ring(ci).dma_start(
                        out=xt_sb[ci][:, ns0 * SLICE : (ns0 + g) * SLICE],
                        in_=xt_d.ap()[
                            ci * P : (ci + 1) * P, ns0 * SLICE : (ns0 + g) * SLICE
                        ],
                    )

            # DMA issue order per HWDGE ring is FIFO. The first data matmuls
            # need only W's rj=0 column slices plus x slice ns0, so those go
            # out first (interleaved per ring), then the W remainders, the
            # next x slices, and bias (needed by the first evac).
            with tc.high_priority():
                for ci in (0, 2, 1, 3):  # SP gets 0,1; ACT gets 2,3
                    ring(ci).dma_start(
                        out=wt_sb[:, ci * B : ci * B + P],
                        in_=wt_d.ap()[ci * P : (ci + 1) * P, 0:P],
                    )
                    ring(ci).dma_start(
                        out=xt_sb[ci][:, 0 : XGROUPS[0] * SLICE],
                        in_=xt_d.ap()[ci * P : (ci + 1) * P, 0 : XGROUPS[0] * SLICE],
                    )
                for ci in (0, 2, 1, 3):
                    ring(ci).dma_start(
                        out=wt_sb[:, ci * B + P : (ci + 1) * B],
                        in_=wt_d.ap()[ci * P : (ci + 1) * P, P:B],
                    )
            ns0 = XGROUPS[0]
            load_x_group(ns0, XGROUPS[1])
            ns0 += XGROUPS[1]
            nc.sync.dma_start(out=bias_sb, in_=b_d.ap())
            for g in XGROUPS[2:]:
                load_x_group(ns0, g)
                ns0 += g

            # PE warm-up burst: dummy matmuls on a DVE-memset zero tile (no
            # DMA or GpSimd dependency): flips the HAM clock gate to 8/8
            # while the first W/x DMAs are still in flight.
            warm_sb = const_pool.tile([P, SLICE], F16)
            nc.vector.memset(warm_sb, 0.0)
            ps_dummy = psD_pool.tile([P, SLICE], F32)
            dummy_inst = nc.tensor.matmul(
                ps_dummy, warm_sb[:, :P], warm_sb, start=True, stop=True
            )
            for _ in range(WARMUP_MATMULS - 1):
                dummy_inst = nc.tensor.matmul(
                    ps_dummy, warm_sb[:, :P], warm_sb, start=True, stop=True
                )

            first = True
            evac_i = 0
            ns0 = 0
            for g in GROUPS:
                for rj in range(RB):
                    banks = [
                        psY_pool.tile([P, SLICE], F32, tag="ps", name=f"psy{j}")
                        for j in range(g)
                    ]
                    # per-bank consecutive accumulation: bank j takes its 4
                    # c-chunk matmuls back-to-back (interleaving accumulation
                    # groups across banks crashes the exec unit)
                    for j in range(g):
                        for ci in range(CB):
                            mm = nc.tensor.matmul(
                                banks[j],
                                wt_sb[:, ci * B + rj * P : ci * B + (rj + 1) * P],
                                xt_sb[ci][:, (ns0 + j) * SLICE : (ns0 + j + 1) * SLICE],
                                start=(ci == 0),
                                stop=(ci == CB - 1),
                            )
                            if first:
                                add_dep_helper(
                                    mm.ins, dummy_inst.ins, sync=False,
                                    reason="warmup before first matmul",
                                )
                                first = False
                    # fused bias add + fp16 cast on the PSUM->SBUF evac,
                    # alternating DVE/ACT per bank
                    for j in range(g):
                        dst = yt_sb[rj][:, (ns0 + j) * SLICE : (ns0 + j + 1) * SLICE]
                        if evac_i % 2 == 0:
                            nc.vector.tensor_scalar_add(
                                dst, banks[j], bias_sb[:, rj : rj + 1]
                            )
                        else:
                            nc.scalar.add(dst, banks[j], bias_sb[:, rj : rj + 1])
                        evac_i += 1
                    # one store per (group, rj); queues behind the x loads in
                    # this ring's FIFO, so it never delays them
                    ring(rj).dma_start(
                        out=y_d.ap()[
                            rj * P : (rj + 1) * P, ns0 * SLICE : (ns0 + g) * SLICE
                        ],
                        in_=yt_sb[rj][:, ns0 * SLICE : (ns0 + g) * SLICE],
                    )
                ns0 += g

    return nc


def _split_pe_multiwaits(nc):
    """Hoist extra sync waits off engine instructions onto sequencer NoOps.

    This walrus build supports only a single attached sync wait per
    instruction; codegen fails with "Too many sync wait commands" otherwise.
    A wait-carrying NoOp immediately before the instruction on the same
    sequencer is semantically identical (the sequencer executes in order).
    """
    k = 0
    for f in nc.m.functions:
        for blk in f.blocks:
            out = []
            changed = False
            for inst in blk.instructions:
                si = inst.sync_info
                if si is not None and len(si.on_wait) > 1:
                    waits = list(si.on_wait)
                    for w in waits[:-1]:
                        nop = mybir.InstNoOp(
                            name=f"I-waitsplit-{k}", ins=[], outs=[]
                        )
                        k += 1
                        nop.engine = inst.engine
                        nop.sync_info = mybir.SyncInfo(on_wait=[w], on_update=[])
                        out.append(nop)
                    inst.sync_info = mybir.SyncInfo(
                        on_wait=[waits[-1]], on_update=list(si.on_update)
                    )
                    changed = True
                out.append(inst)
            if changed:
                blk.instructions = out
    return nc


def _get_nc():
    if "nc" not in _CACHE:
        _CACHE["nc"] = _split_pe_multiwaits(_build_bass())
    return _CACHE["nc"]


def _run(inputs, trace=False):
    x = np.ascontiguousarray(np.asarray(inputs["x"], dtype=np.float32))
    w = np.ascontiguousarray(np.asarray(inputs["weight_blocks"], dtype=np.float32))
    bias = np.ascontiguousarray(np.asarray(inputs["bias"], dtype=np.float32))
    assert x.shape == (N, D) and w.shape == (NB, B, B) and bias.shape == (D,)
    nc = _get_nc()
    x16 = x.astype(np.float16)
    in_maps = [
        {
            "xt": np.ascontiguousarray(x16[:, k * B : (k + 1) * B].T),
            "wt": np.ascontiguousarray(w[k].T.astype(np.float16)),
            "b": np.ascontiguousarray(
                bias[k * B : (k + 1) * B].reshape(RB, P).T
            ),
        }
        for k in range(NB)
    ]
    try:
        res = run_bass_kernel_spmd(
            nc, in_maps, core_ids=list(range(NB)), trace=trace
        )
    except Exception:
        # the axon-tunneled devices occasionally report a transient
        # NRT_EXEC_UNIT_UNRECOVERABLE; a single retry has always recovered
        res = run_bass_kernel_spmd(
            nc, in_maps, core_ids=list(range(NB)), trace=trace
        )
    y = np.empty((N, D), dtype=np.float32)
    for k in range(NB):
        y[:, k * B : (k + 1) * B] = res.results[k]["y"].T
    return y, res


def kernel(**inputs):
    y, _ = _run(inputs, trace=False)
    return y


def kernel_traced(**inputs):
    return _run(inputs, trace=True)
